# revision 30
# baseline (speedup 1.0000x reference)
"""Trainium2 Bass kernel for dilated window attention (nn_Dilated_attn).

Strategy (8 NeuronCores, data-parallel over the 1024 dilated windows):
 - Host regroups x into (1024 windows, 64 tok, 512) and shards 128 windows/core.
 - RoPE is folded into the QKV weights: 8 row-rotation + 8 col-rotation weight
   variants per q/k half, applied via position-sliced GEMMs (tokens sharing a
   within-window row/col). Head dims are de-interleaved (dot-product invariant).
 - q.T/k.T computed d-major (dims on partitions) = the layout attention needs.
   v computed token-major. All matmuls bf16, fp32 PSUM accumulate.
 - Scores computed transposed per (2-window, head) with 4-head PE row-packing;
   batched exp on the scalar engine (no max subtraction; scores ~N(0,1)).
 - AV uses exp(S).T as the stationary operand with an appended ones-column in v
   to produce softmax denominators per q-partition; normalize via one
   broadcast-AP multiply. PE-transpose -> d-major proj GEMM -> DMA out.
"""

import sys
import numpy as np
import ml_dtypes

sys.path.insert(0, "/opt/trn_rl_repo")

import concourse.bass as bass  # noqa: E402
import concourse.tile as tile  # noqa: E402
from concourse import bacc, mybir  # noqa: E402
from concourse.masks import make_identity  # noqa: E402
from contextlib import ExitStack  # noqa: E402

# ---------------- problem constants ----------------
DIM = 512
HEADS = 16
HD = 32
WH, WW = 8, 8
D0, D1 = 2, 2
TWH, TWW = 16, 16
SCALE = HD ** -0.5
N_CORES = 8
NWIN = 1024
NTOK = 64
WIN_PER_CORE = NWIN // N_CORES      # 128
NT = WIN_PER_CORE * NTOK            # 8192 tokens per core

d_half = HD // 2                    # 16
INV = 1.0 / (10000.0 ** (np.arange(0, d_half, 2, dtype=np.float64) / d_half))

BF16 = mybir.dt.bfloat16
F32 = mybir.dt.float32


# ---------------- host-side data prep ----------------

def window_partition(x):
    B, H, W, C = x.shape
    xw = x.reshape(B, H // TWH, TWH, W // TWW, TWW, C).transpose(0, 1, 3, 2, 4, 5)
    xw = xw.reshape(-1, TWH, TWW, C)
    B_ = xw.shape[0]
    xw = xw.reshape(B_, TWH // D0, D0, TWW // D1, D1, C).transpose(0, 1, 3, 2, 4, 5)
    xw = xw.reshape(B_, WH * WW, D0 * D1, C)
    return xw.transpose(0, 2, 1, 3).reshape(B_ * D0 * D1, WH * WW, C)


def window_unpartition(ow, B, H, W):
    C = ow.shape[-1]
    B_ = ow.shape[0] // (D0 * D1)
    o = ow.reshape(B_, D0 * D1, WH * WW, C).transpose(0, 2, 1, 3)
    o = o.reshape(B_, WH, WW, D0, D1, C).transpose(0, 1, 3, 2, 4, 5)
    o = o.reshape(B_, TWH, TWW, C)
    nh, nw = H // TWH, W // TWW
    o = o.reshape(B, nh, nw, TWH, TWW, C).transpose(0, 1, 3, 2, 4, 5)
    return o.reshape(B, H, W, C)


PERM32 = np.concatenate([
    np.arange(0, d_half, 2), np.arange(1, d_half, 2),
    d_half + np.arange(0, d_half, 2), d_half + np.arange(1, d_half, 2),
])


def _rot_mat(theta_vec):
    c, s = np.cos(theta_vec), np.sin(theta_vec)
    R = np.zeros((16, 16))
    R[np.arange(8), np.arange(8)] = c
    R[np.arange(8), 8 + np.arange(8)] = -s
    R[8 + np.arange(8), np.arange(8)] = s
    R[8 + np.arange(8), 8 + np.arange(8)] = c
    return R


def prep_weights(qkv_w, qkv_b, proj_w, proj_b):
    qkv_w = np.asarray(qkv_w, np.float64)
    qkv_b = np.asarray(qkv_b, np.float64)
    proj_w = np.asarray(proj_w, np.float64)
    proj_b = np.asarray(proj_b, np.float64)

    Wq = qkv_w[:DIM] * SCALE
    Wk = qkv_w[DIM:2 * DIM]
    Wv = qkv_w[2 * DIM:]
    bq = qkv_b[:DIM] * SCALE
    bk = qkv_b[DIM:2 * DIM]
    bv = qkv_b[2 * DIM:]

    perm = (np.arange(HEADS)[:, None] * HD + PERM32[None, :]).reshape(-1)
    Wq_p, bq_p = Wq[perm], bq[perm]
    Wk_p, bk_p = Wk[perm], bk[perm]

    idx = np.arange(DIM).reshape(HEADS, HD)
    a_rows = idx[:, :16].reshape(-1)
    b_rows = idx[:, 16:].reshape(-1)

    def variants(Wp, bp, rows):
        Wh, bh = Wp[rows], bp[rows]
        Ws, bs = [], []
        for t in range(8):
            R = np.kron(np.eye(HEADS), _rot_mat(t * INV))
            Ws.append(R @ Wh)
            bs.append(R @ bh)
        return np.stack(Ws), np.stack(bs)

    Wqa, bqa = variants(Wq_p, bq_p, a_rows)
    Wqb, bqb = variants(Wq_p, bq_p, b_rows)
    Wka, bka = variants(Wk_p, bk_p, a_rows)
    Wkb, bkb = variants(Wk_p, bk_p, b_rows)

    proj_b_eff = proj_w @ bv + proj_b
    bias_zero = (np.abs(np.concatenate([bqa, bqb, bka, bkb], None)).max() == 0.0
                 and np.abs(proj_b_eff).max() == 0.0)

    def pack_lhsT(Wvar):  # (8, 256, 512) -> (8, 4, 128, 256) bf16
        WT = Wvar.transpose(0, 2, 1)                # (8, 512, 256)
        return np.ascontiguousarray(
            WT.reshape(8, 4, 128, 256)).astype(ml_dtypes.bfloat16)

    return dict(
        wqa=pack_lhsT(Wqa), wqb=pack_lhsT(Wqb),
        wka=pack_lhsT(Wka), wkb=pack_lhsT(Wkb),
        wv=np.ascontiguousarray(Wv.T.reshape(4, 128, 512)).astype(ml_dtypes.bfloat16),
        wp=np.ascontiguousarray(proj_w.T.reshape(4, 128, 512)).astype(ml_dtypes.bfloat16),
        proj_b_eff=proj_b_eff.astype(np.float32),
        bias_zero=bias_zero,
    )


# ---------------- device program ----------------

def build_program(CH=8, NCHUNK=16, debug_stop=None):
    """One-core SPMD program. CH windows per chunk, NCHUNK chunks.
    debug_stop: one of qkgemm|scatter|vprime|scores|av|norm|trans to truncate
    the pipeline after that stage and DMA the stage output to a `dbg` tensor."""
    nt = CH * NCHUNK * NTOK        # tokens per core
    TPC = CH * NTOK                # tokens per chunk
    NG = CH // 2                   # 2-window groups per chunk
    PW = min(512, TPC)             # proj store width

    nc = bacc.Bacc(trn_type="TRN2", target_bir_lowering=False, debug=False)

    xt_d = nc.dram_tensor("xt", [4, 128, nt], BF16, kind="ExternalInput").ap()
    w_d = {}
    for nm in ("wqa", "wqb", "wka", "wkb"):
        w_d[nm] = nc.dram_tensor(nm, [8, 4, 128, 256], BF16,
                                 kind="ExternalInput").ap()
    wv_d = nc.dram_tensor("wv", [4, 128, 512], BF16, kind="ExternalInput").ap()
    wp_d = nc.dram_tensor("wp", [4, 128, 512], BF16, kind="ExternalInput").ap()
    out_d = nc.dram_tensor("outT", [4, 128, nt], F32, kind="ExternalOutput").ap()

    with tile.TileContext(nc) as tc, ExitStack() as ctx:
        const_p = ctx.enter_context(tc.tile_pool(name="const", bufs=1))
        w_p = ctx.enter_context(tc.tile_pool(name="weights", bufs=1))
        xt_p = ctx.enter_context(tc.tile_pool(name="xt", bufs=2))
        stag_p = ctx.enter_context(tc.tile_pool(name="stag", bufs=1))
        qkT_p = ctx.enter_context(tc.tile_pool(name="qkT", bufs=2))
        vp_p = ctx.enter_context(tc.tile_pool(name="vp", bufs=2))
        exp_p = ctx.enter_context(tc.tile_pool(name="exp", bufs=1))
        ao_p = ctx.enter_context(tc.tile_pool(name="ao", bufs=2))
        aoT_p = ctx.enter_context(tc.tile_pool(name="aoT", bufs=1))
        rc_p = ctx.enter_context(tc.tile_pool(name="rc", bufs=2))
        os_p = ctx.enter_context(tc.tile_pool(name="os", bufs=2))

        ps_gemm = ctx.enter_context(tc.tile_pool(name="ps_gemm", bufs=2, space="PSUM"))
        ps_sc = ctx.enter_context(tc.tile_pool(name="ps_sc", bufs=1, space="PSUM"))
        ps_av = ctx.enter_context(tc.tile_pool(name="ps_av", bufs=1, space="PSUM"))
        ps_tr = ctx.enter_context(tc.tile_pool(name="ps_tr", bufs=1, space="PSUM"))
        ps_pj = ctx.enter_context(tc.tile_pool(name="ps_pj", bufs=1, space="PSUM"))

        ident = const_p.tile([128, 128], BF16)
        make_identity(nc, ident[:])
        # per-window ones columns: col ws = 1 on that window's 64 rows
        wsones = const_p.tile([128, 2], BF16)
        nc.gpsimd.memset(wsones[:], 0.0)
        nc.gpsimd.memset(wsones[0:64, 0:1], 1.0)
        nc.gpsimd.memset(wsones[64:128, 1:2], 1.0)

        # resident weights
        w_sb = {}
        for nm in ("wqa", "wqb", "wka", "wkb"):
            t = w_p.tile([128, 8, 4, 256], BF16, tag=nm)
            nc.sync.dma_start(out=t[:], in_=w_d[nm].rearrange("v k p m -> p v k m"))
            w_sb[nm] = t
        wv_sb = w_p.tile([128, 4, 512], BF16, tag="wv")
        nc.sync.dma_start(out=wv_sb[:], in_=wv_d.rearrange("k p n -> p k n"))
        wp_sb = w_p.tile([128, 4, 512], BF16, tag="wp")
        nc.sync.dma_start(out=wp_sb[:], in_=wp_d.rearrange("k p n -> p k n"))

        for ck in range(NCHUNK):
            t0 = ck * TPC

            # ---- load x.T chunk ----
            xt_t = xt_p.tile([128, 4, TPC], BF16, tag="xt")
            nc.sync.dma_start(
                out=xt_t[:],
                in_=xt_d[:, :, t0:t0 + TPC].rearrange("k p t -> p k t"))
            xt4 = xt_t[:].rearrange("p k (w r c) -> p k w r c", w=CH, r=8, c=8)

            # ---- q/k GEMMs (rope folded), into staging ----
            stag = stag_p.tile([128, 8, TPC], BF16, tag="stag")
            for Ti, T in enumerate("qk"):
                for Hi, half in enumerate("ab"):
                    wt = w_sb["w" + T + half]
                    for Mc in range(2):
                        blk = (Ti * 2 + Hi) * 2 + Mc
                        if half == "a":
                            dst4 = stag[:, blk].rearrange(
                                "p (w r c) -> p r w c", w=CH, r=8, c=8)
                        else:
                            dst4 = stag[:, blk].rearrange(
                                "p (w r c) -> p c w r", w=CH, r=8, c=8)
                        NW = CH * 8
                        for vg in range(2):
                            ps = ps_gemm.tile([128, 512], F32, tag="gemm")
                            for vv in range(4):
                                v8 = 4 * vg + vv
                                if half == "a":
                                    rhs = xt4[:, :, :, v8, :]   # p k w c
                                else:
                                    rhs = xt4[:, :, :, :, v8]   # p k w r
                                for kc in range(4):
                                    nc.tensor.matmul(
                                        ps[:, NW * vv:NW * (vv + 1)],
                                        lhsT=wt[:, v8, kc, 128 * Mc:128 * Mc + 128],
                                        rhs=rhs[:, kc],
                                        start=(vv == 0 and kc == 0),
                                        stop=(vv == 3 and kc == 3))
                            nc.vector.tensor_copy(
                                out=dst4[:, 4 * vg:4 * vg + 4],
                                in_=ps[:, 0:4 * NW].rearrange(
                                    "p (v w c) -> p v w c", v=4, w=CH))

            if debug_stop == "qkgemm":
                dbg = nc.dram_tensor("dbg", [128, 8, TPC], BF16,
                                     kind="ExternalOutput").ap()
                nc.sync.dma_start(out=dbg, in_=stag[:])
                break

            # ---- scatter staging -> head-major qT/kT at partitions 0:32 ----
            # Every head's 32 grouped d-rows land on partitions 0:32 (a-half
            # rows 0:16, b-half 16:32) at free offset h*TPC, so every scores
            # matmul reads its stationary from the same partition base.
            # (Cycling partial-row stationary bases across matmuls crashes
            # the device; fixed-base partial-row stationaries are fine.)
            qkT = {}
            for Ti, T in enumerate("qk"):
                qt = qkT_p.tile([128, 16, TPC], BF16, tag=T + "T")
                qkT[T] = qt
                for Mc in range(2):
                    for Hi in range(2):          # a, b halves
                        blk = (Ti * 2 + Hi) * 2 + Mc
                        for hl in range(8):
                            src = stag[16 * hl:16 * hl + 16, blk]
                            dst = qt[16 * Hi:16 * Hi + 16, 8 * Mc + hl, :]
                            nc.sync.dma_start(out=dst, in_=src)

            if debug_stop == "scatter":
                dbg = nc.dram_tensor("dbg", [2, 128, 16, TPC], BF16,
                                     kind="ExternalOutput").ap()
                nc.sync.dma_start(out=dbg[0], in_=qkT["q"][:])
                nc.sync.dma_start(out=dbg[1], in_=qkT["k"][:])
                break

            # ---- v GEMM (token-major) -> zero-padded per-window v ----
            # vz[:, g, ws, h, :] holds head h's v rows for window ws of group
            # g on that window's 64 partitions, zeros on the other 64. This
            # keeps every AV stationary full-128-partition (partial-row
            # stationaries alternating base partitions crash the device).
            vz = vp_p.tile([128, NG, 2, 16, 32], BF16, tag="vp")
            nc.gpsimd.memset(vz[64:128, :, 0], 0.0)
            nc.gpsimd.memset(vz[0:64, :, 1], 0.0)
            for g in range(NG):
                ps = ps_gemm.tile([128, 512], F32, tag="gemm")
                for kc in range(4):
                    nc.tensor.matmul(
                        ps[:],
                        lhsT=xt_t[:, kc, 128 * g:128 * (g + 1)],
                        rhs=wv_sb[:, kc],
                        start=(kc == 0), stop=(kc == 3))
                psh = ps[:].rearrange("p (h e) -> p h e", h=16)
                nc.vector.tensor_copy(out=vz[0:64, g, 0], in_=psh[0:64])
                nc.vector.tensor_copy(out=vz[64:128, g, 1], in_=psh[64:128])

            if debug_stop == "vprime":
                dbg = nc.dram_tensor("dbg", [128, NG, 2, 16, 32], BF16,
                                     kind="ExternalOutput").ap()
                nc.sync.dma_start(out=dbg, in_=vz[:])
                break

            # ---- attention per (2-window group g, 8-head group G) ----
            ao = ao_p.tile([128, NG, 512], BF16, tag="ao")
            for g in range(NG):
                expg = exp_p.tile([128, 2, 8, 128], BF16, tag="exp")
                rcp = rc_p.tile([128, 2, 16], F32, tag="rc")
                for G in range(2):
                    # scores.T : 8 heads into one (128, 1024) psum (2 banks)
                    ps_s = ps_sc.tile([128, 8, 128], F32, tag="sc")
                    for jj in range(8):
                        h = 8 * G + jj
                        cols = slice(128 * g, 128 * (g + 1))
                        nc.tensor.matmul(
                            ps_s[:, jj],
                            lhsT=qkT["k"][0:32, h, cols],
                            rhs=qkT["q"][0:32, h, cols],
                            start=(jj % 4 == 0), stop=(jj % 4 == 3))
                    nc.scalar.activation(
                        out=expg[:, G], in_=ps_s[:],
                        func=mybir.ActivationFunctionType.Exp)
                    if debug_stop == "scores":
                        continue

                    # AV + denominators. Stationary = exp(S.T) full 128
                    # partitions (both windows' keys); the zero-padded vz /
                    # wsones right operands mask the other window's rows.
                    # Bank 0: 16 v-slots of 32; bank 1: denominator pairs.
                    ps_a = ps_av.tile([128, 2, 512], F32, tag="av")
                    for jj in range(8):
                        h = 8 * G + jj
                        for ws in range(2):
                            s = jj * 2 + ws
                            nc.tensor.matmul(
                                ps_a[:, 0, 32 * s:32 * s + 32],
                                lhsT=expg[:, G, jj],
                                rhs=vz[:, g, ws, h],
                                start=(s == 0), stop=(s == 15))
                        nc.tensor.matmul(
                            ps_a[:, 1, 2 * jj:2 * jj + 2],
                            lhsT=expg[:, G, jj],
                            rhs=wsones[:],
                            start=(jj == 0), stop=(jj == 7))

                    if debug_stop == "av":
                        avdump = ao_p.tile([128, 2, 512], F32, tag="avdump")
                        nc.vector.tensor_copy(out=avdump[:], in_=ps_a[:])
                        continue

                    # normalize: recip of denominators, scale valid halves
                    vslots = ps_a[:, 0, :].rearrange(
                        "p (j w e) -> p j w e", j=8, w=2)
                    dens = ps_a[:, 1, 0:16].rearrange("p (j w) -> p j w", j=8)
                    nc.vector.reciprocal(
                        out=rcp[:, G].rearrange("p (j w) -> p j w", j=8),
                        in_=dens)
                    if debug_stop == "recip":
                        continue
                    for ws in range(2):
                        src = vslots[64 * ws:64 * ws + 64, :, ws, :]
                        rin = rcp[64 * ws:64 * ws + 64, G] \
                            .rearrange("p (j w) -> p j w", j=8)[:, :, ws] \
                            .broadcast_to((64, 8, 32))
                        dst = ao[64 * ws:64 * ws + 64, g,
                                 256 * G:256 * (G + 1)].rearrange(
                            "p (j e) -> p j e", j=8)
                        nc.vector.tensor_tensor(
                            out=dst, in0=src, in1=rin, op=mybir.AluOpType.mult)

            if debug_stop in ("scores", "norm", "av", "recip"):
                if debug_stop == "norm":
                    dbg = nc.dram_tensor("dbg", [128, NG, 512], BF16,
                                         kind="ExternalOutput").ap()
                    nc.sync.dma_start(out=dbg, in_=ao[:])
                elif debug_stop == "scores":
                    dbg = nc.dram_tensor("dbg", [128, 2, 8, 128], BF16,
                                         kind="ExternalOutput").ap()
                    nc.sync.dma_start(out=dbg, in_=expg[:])
                elif debug_stop == "recip":
                    dbg = nc.dram_tensor("dbg", [128, 2, 16], F32,
                                         kind="ExternalOutput").ap()
                    nc.sync.dma_start(out=dbg, in_=rcp[:])
                else:
                    dbg = nc.dram_tensor("dbg", [128, 2, 512], F32,
                                         kind="ExternalOutput").ap()
                    nc.sync.dma_start(out=dbg, in_=avdump[:])
                break

            # ---- transpose attnout -> d-major ----
            aoT = aoT_p.tile([128, 4, TPC], BF16, tag="aoT")
            for g in range(NG):
                ps_t = ps_tr.tile([128, 4, 128], BF16, tag="tr")
                for m in range(4):
                    nc.tensor.transpose(
                        ps_t[:, m],
                        ao[:, g, 128 * m:128 * (m + 1)],
                        ident[:])
                nc.vector.tensor_copy(
                    out=aoT[:, :, 128 * g:128 * (g + 1)], in_=ps_t[:])

            if debug_stop == "trans":
                dbg = nc.dram_tensor("dbg", [128, 4, TPC], BF16,
                                     kind="ExternalOutput").ap()
                nc.sync.dma_start(out=dbg, in_=aoT[:])
                break

            # ---- proj GEMM (d-major out) + store ----
            for Mc in range(4):
                for th in range(TPC // PW):
                    ps_o = ps_pj.tile([128, PW], F32, tag="pj")
                    for m in range(4):
                        nc.tensor.matmul(
                            ps_o[:],
                            lhsT=wp_sb[:, m, 128 * Mc:128 * Mc + 128],
                            rhs=aoT[:, m, PW * th:PW * (th + 1)],
                            start=(m == 0), stop=(m == 3))
                    ost = os_p.tile([128, PW], F32, tag="os")
                    nc.scalar.copy(out=ost[:], in_=ps_o[:])
                    nc.sync.dma_start(
                        out=out_d[Mc, :, t0 + PW * th:t0 + PW * (th + 1)],
                        in_=ost[:])

    nc.compile()
    return nc


# ---------------- host driver ----------------

_PROG_CACHE = {}


def _get_program(CH=8, NCHUNK=16):
    key = (CH, NCHUNK)
    if key not in _PROG_CACHE:
        _PROG_CACHE[key] = build_program(CH, NCHUNK)
    return _PROG_CACHE[key]


def make_in_maps(x, wp_dict, CH=8, NCHUNK=16, n_cores=N_CORES):
    xw = window_partition(np.asarray(x, np.float32))     # (1024, 64, 512)
    nt = CH * NCHUNK * NTOK
    win_per_core = nt // NTOK
    in_maps = []
    for c in range(n_cores):
        xs = xw[c * win_per_core:(c + 1) * win_per_core].reshape(nt, DIM)
        xt = np.ascontiguousarray(xs.T).astype(ml_dtypes.bfloat16)
        in_maps.append(dict(
            xt=np.ascontiguousarray(xt.reshape(4, 128, nt)),
            wqa=wp_dict["wqa"], wqb=wp_dict["wqb"],
            wka=wp_dict["wka"], wkb=wp_dict["wkb"],
            wv=wp_dict["wv"], wp=wp_dict["wp"],
        ))
    return in_maps


def _run(x, qkv_w, qkv_b, proj_w, proj_b, trace=False):
    from concourse.bass_utils import run_bass_kernel_spmd

    wp_dict = prep_weights(qkv_w, qkv_b, proj_w, proj_b)
    assert wp_dict["bias_zero"], "nonzero biases not supported by this kernel"

    nc = _get_program()
    in_maps = make_in_maps(x, wp_dict)
    res = run_bass_kernel_spmd(nc, in_maps, list(range(N_CORES)), trace=trace)

    x = np.asarray(x)
    B, H, W, C = x.shape
    outs = []
    for c in range(N_CORES):
        oT = np.asarray(res.results[c]["outT"]).reshape(DIM, NT)
        outs.append(np.ascontiguousarray(oT.T))          # (8192, 512)
    ow = np.concatenate(outs, 0).reshape(NWIN, NTOK, DIM)
    out = window_unpartition(ow, B, H, W).astype(np.float32)
    return out, res


def kernel(x, qkv_w, qkv_b, proj_w, proj_b):
    out, _ = _run(x, qkv_w, qkv_b, proj_w, proj_b, trace=False)
    return out


if __name__ == "__main__":
    build_program(2, 2)
    print("mini program built OK")
    build_program()
    print("full program built OK")


# revision 37
# speedup vs baseline: 1.3712x; 1.3712x over previous
"""Trainium2 Bass kernel for dilated window attention (nn_Dilated_attn).

Strategy (8 NeuronCores, data-parallel over the 1024 dilated windows):
 - Host regroups x into (1024 windows, 64 tok, 512) and shards 128 windows/core.
 - RoPE is folded into the QKV weights: 8 row-rotation + 8 col-rotation weight
   variants per q/k half, applied via position-sliced GEMMs (tokens sharing a
   within-window row/col). Head dims are de-interleaved (dot-product invariant).
 - q.T/k.T computed d-major (dims on partitions) = the layout attention needs.
   v computed token-major. All matmuls bf16, fp32 PSUM accumulate.
 - Scores computed transposed per (2-window, head) with 4-head PE row-packing;
   batched exp on the scalar engine (no max subtraction; scores ~N(0,1)).
 - AV uses exp(S).T as the stationary operand with an appended ones-column in v
   to produce softmax denominators per q-partition; normalize via one
   broadcast-AP multiply. PE-transpose -> d-major proj GEMM -> DMA out.
"""

import sys
import numpy as np
import ml_dtypes

sys.path.insert(0, "/opt/trn_rl_repo")

import concourse.bass as bass  # noqa: E402
import concourse.tile as tile  # noqa: E402
from concourse import bacc, mybir  # noqa: E402
from concourse.masks import make_identity  # noqa: E402
from contextlib import ExitStack  # noqa: E402

# ---------------- problem constants ----------------
DIM = 512
HEADS = 16
HD = 32
WH, WW = 8, 8
D0, D1 = 2, 2
TWH, TWW = 16, 16
SCALE = HD ** -0.5
N_CORES = 8
NWIN = 1024
NTOK = 64
WIN_PER_CORE = NWIN // N_CORES      # 128
NT = WIN_PER_CORE * NTOK            # 8192 tokens per core

d_half = HD // 2                    # 16
INV = 1.0 / (10000.0 ** (np.arange(0, d_half, 2, dtype=np.float64) / d_half))

BF16 = mybir.dt.bfloat16
F32 = mybir.dt.float32


# ---------------- host-side data prep ----------------

def window_partition(x):
    B, H, W, C = x.shape
    xw = x.reshape(B, H // TWH, TWH, W // TWW, TWW, C).transpose(0, 1, 3, 2, 4, 5)
    xw = xw.reshape(-1, TWH, TWW, C)
    B_ = xw.shape[0]
    xw = xw.reshape(B_, TWH // D0, D0, TWW // D1, D1, C).transpose(0, 1, 3, 2, 4, 5)
    xw = xw.reshape(B_, WH * WW, D0 * D1, C)
    return xw.transpose(0, 2, 1, 3).reshape(B_ * D0 * D1, WH * WW, C)


def window_unpartition(ow, B, H, W):
    C = ow.shape[-1]
    B_ = ow.shape[0] // (D0 * D1)
    o = ow.reshape(B_, D0 * D1, WH * WW, C).transpose(0, 2, 1, 3)
    o = o.reshape(B_, WH, WW, D0, D1, C).transpose(0, 1, 3, 2, 4, 5)
    o = o.reshape(B_, TWH, TWW, C)
    nh, nw = H // TWH, W // TWW
    o = o.reshape(B, nh, nw, TWH, TWW, C).transpose(0, 1, 3, 2, 4, 5)
    return o.reshape(B, H, W, C)


PERM32 = np.concatenate([
    np.arange(0, d_half, 2), np.arange(1, d_half, 2),
    d_half + np.arange(0, d_half, 2), d_half + np.arange(1, d_half, 2),
])


def _rot_mat(theta_vec):
    c, s = np.cos(theta_vec), np.sin(theta_vec)
    R = np.zeros((16, 16))
    R[np.arange(8), np.arange(8)] = c
    R[np.arange(8), 8 + np.arange(8)] = -s
    R[8 + np.arange(8), np.arange(8)] = s
    R[8 + np.arange(8), 8 + np.arange(8)] = c
    return R


def prep_weights(qkv_w, qkv_b, proj_w, proj_b):
    qkv_w = np.asarray(qkv_w, np.float64)
    qkv_b = np.asarray(qkv_b, np.float64)
    proj_w = np.asarray(proj_w, np.float64)
    proj_b = np.asarray(proj_b, np.float64)

    Wq = qkv_w[:DIM] * SCALE
    Wk = qkv_w[DIM:2 * DIM]
    Wv = qkv_w[2 * DIM:]
    bq = qkv_b[:DIM] * SCALE
    bk = qkv_b[DIM:2 * DIM]
    bv = qkv_b[2 * DIM:]

    perm = (np.arange(HEADS)[:, None] * HD + PERM32[None, :]).reshape(-1)
    Wq_p, bq_p = Wq[perm], bq[perm]
    Wk_p, bk_p = Wk[perm], bk[perm]

    idx = np.arange(DIM).reshape(HEADS, HD)
    a_rows = idx[:, :16].reshape(-1)
    b_rows = idx[:, 16:].reshape(-1)

    def variants(Wp, bp, rows):
        Wh, bh = Wp[rows], bp[rows]
        Ws, bs = [], []
        for t in range(8):
            R = np.kron(np.eye(HEADS), _rot_mat(t * INV))
            Ws.append(R @ Wh)
            bs.append(R @ bh)
        return np.stack(Ws), np.stack(bs)

    Wqa, bqa = variants(Wq_p, bq_p, a_rows)
    Wqb, bqb = variants(Wq_p, bq_p, b_rows)
    Wka, bka = variants(Wk_p, bk_p, a_rows)
    Wkb, bkb = variants(Wk_p, bk_p, b_rows)

    proj_b_eff = proj_w @ bv + proj_b
    bias_zero = (np.abs(np.concatenate([bqa, bqb, bka, bkb], None)).max() == 0.0
                 and np.abs(proj_b_eff).max() == 0.0)

    def pack_lhsT(Wvar):  # (8, 256, 512) -> (8, 4, 128, 256) bf16
        WT = Wvar.transpose(0, 2, 1)                # (8, 512, 256)
        return np.ascontiguousarray(
            WT.reshape(8, 4, 128, 256)).astype(ml_dtypes.bfloat16)

    return dict(
        wqa=pack_lhsT(Wqa), wqb=pack_lhsT(Wqb),
        wka=pack_lhsT(Wka), wkb=pack_lhsT(Wkb),
        wv=np.ascontiguousarray(Wv.T.reshape(4, 128, 512)).astype(ml_dtypes.bfloat16),
        wp=np.ascontiguousarray(proj_w.T.reshape(4, 128, 512)).astype(ml_dtypes.bfloat16),
        proj_b_eff=proj_b_eff.astype(np.float32),
        bias_zero=bias_zero,
    )


# ---------------- device program ----------------

def build_program(CH=8, NCHUNK=16, debug_stop=None):
    """One-core SPMD program. CH windows per chunk, NCHUNK chunks.
    debug_stop: one of qkgemm|scatter|vprime|scores|av|norm|trans to truncate
    the pipeline after that stage and DMA the stage output to a `dbg` tensor."""
    nt = CH * NCHUNK * NTOK        # tokens per core
    TPC = CH * NTOK                # tokens per chunk
    NG = CH // 2                   # 2-window groups per chunk
    PW = min(512, TPC)             # proj store width

    nc = bacc.Bacc(trn_type="TRN2", target_bir_lowering=False, debug=False)

    xt_d = nc.dram_tensor("xt", [4, 128, nt], BF16, kind="ExternalInput").ap()
    w_d = {}
    for nm in ("wqa", "wqb", "wka", "wkb"):
        w_d[nm] = nc.dram_tensor(nm, [8, 4, 128, 256], BF16,
                                 kind="ExternalInput").ap()
    wv_d = nc.dram_tensor("wv", [4, 128, 512], BF16, kind="ExternalInput").ap()
    wp_d = nc.dram_tensor("wp", [4, 128, 512], BF16, kind="ExternalInput").ap()
    out_d = nc.dram_tensor("outT", [4, 128, nt], F32, kind="ExternalOutput").ap()

    with tile.TileContext(nc) as tc, ExitStack() as ctx:
        const_p = ctx.enter_context(tc.tile_pool(name="const", bufs=1))
        w_p = ctx.enter_context(tc.tile_pool(name="weights", bufs=1))
        xt_p = ctx.enter_context(tc.tile_pool(name="xt", bufs=2))
        stag_p = ctx.enter_context(tc.tile_pool(name="stag", bufs=1))
        qkT_p = ctx.enter_context(tc.tile_pool(name="qkT", bufs=2))
        vp_p = ctx.enter_context(tc.tile_pool(name="vp", bufs=1))
        exp_p = ctx.enter_context(tc.tile_pool(name="exp", bufs=3))
        ao_p = ctx.enter_context(tc.tile_pool(name="ao", bufs=2))
        aoT_p = ctx.enter_context(tc.tile_pool(name="aoT", bufs=1))
        rc_p = ctx.enter_context(tc.tile_pool(name="rc", bufs=2))
        os_p = ctx.enter_context(tc.tile_pool(name="os", bufs=2))

        ps_gemm = ctx.enter_context(tc.tile_pool(name="ps_gemm", bufs=2, space="PSUM"))
        ps_sc = ctx.enter_context(tc.tile_pool(name="ps_sc", bufs=2, space="PSUM"))
        ps_av = ctx.enter_context(tc.tile_pool(name="ps_av", bufs=2, space="PSUM"))
        ps_tr = ctx.enter_context(tc.tile_pool(name="ps_tr", bufs=1, space="PSUM"))
        ps_pj = ctx.enter_context(tc.tile_pool(name="ps_pj", bufs=1, space="PSUM"))

        ident = const_p.tile([128, 128], BF16)
        make_identity(nc, ident[:])
        # per-window ones columns: col ws = 1 on that window's 64 rows
        wsones = const_p.tile([128, 2], BF16)
        nc.gpsimd.memset(wsones[:], 0.0)
        nc.gpsimd.memset(wsones[0:64, 0:1], 1.0)
        nc.gpsimd.memset(wsones[64:128, 1:2], 1.0)

        # resident weights
        w_sb = {}
        for nm in ("wqa", "wqb", "wka", "wkb"):
            t = w_p.tile([128, 8, 4, 256], BF16, tag=nm)
            nc.sync.dma_start(out=t[:], in_=w_d[nm].rearrange("v k p m -> p v k m"))
            w_sb[nm] = t
        wv_sb = w_p.tile([128, 4, 512], BF16, tag="wv")
        nc.sync.dma_start(out=wv_sb[:], in_=wv_d.rearrange("k p n -> p k n"))
        wp_sb = w_p.tile([128, 4, 512], BF16, tag="wp")
        nc.sync.dma_start(out=wp_sb[:], in_=wp_d.rearrange("k p n -> p k n"))

        # persistent zero-padded v tiles (zeros written once, data per chunk)
        vz_tiles = []
        for i in range(2):
            vzt = vp_p.tile([128, NG, 2, 16, 32], BF16, tag=f"vp{i}")
            nc.gpsimd.memset(vzt[64:128, :, 0], 0.0)
            nc.gpsimd.memset(vzt[0:64, :, 1], 0.0)
            vz_tiles.append(vzt)

        for ck in range(NCHUNK):
            t0 = ck * TPC

            # ---- load x.T chunk ----
            xt_t = xt_p.tile([128, 4, TPC], BF16, tag="xt")
            nc.sync.dma_start(
                out=xt_t[:],
                in_=xt_d[:, :, t0:t0 + TPC].rearrange("k p t -> p k t"))
            xt4 = xt_t[:].rearrange("p k (w r c) -> p k w r c", w=CH, r=8, c=8)

            # ---- q/k GEMMs (rope folded), into staging ----
            stag = stag_p.tile([128, 8, TPC], BF16, tag="stag")
            for Ti, T in enumerate("qk"):
                for Hi, half in enumerate("ab"):
                    wt = w_sb["w" + T + half]
                    for Mc in range(2):
                        blk = (Ti * 2 + Hi) * 2 + Mc
                        if half == "a":
                            dst4 = stag[:, blk].rearrange(
                                "p (w r c) -> p r w c", w=CH, r=8, c=8)
                        else:
                            dst4 = stag[:, blk].rearrange(
                                "p (w r c) -> p c w r", w=CH, r=8, c=8)
                        NW = CH * 8
                        for vg in range(2):
                            ps = ps_gemm.tile([128, 512], F32, tag="gemm")
                            for vv in range(4):
                                v8 = 4 * vg + vv
                                if half == "a":
                                    rhs = xt4[:, :, :, v8, :]   # p k w c
                                else:
                                    rhs = xt4[:, :, :, :, v8]   # p k w r
                                for kc in range(4):
                                    nc.tensor.matmul(
                                        ps[:, NW * vv:NW * (vv + 1)],
                                        lhsT=wt[:, v8, kc, 128 * Mc:128 * Mc + 128],
                                        rhs=rhs[:, kc],
                                        start=(vv == 0 and kc == 0),
                                        stop=(vv == 3 and kc == 3))
                            nc.vector.tensor_copy(
                                out=dst4[:, 4 * vg:4 * vg + 4],
                                in_=ps[:, 0:4 * NW].rearrange(
                                    "p (v w c) -> p v w c", v=4, w=CH))

            if debug_stop == "qkgemm":
                dbg = nc.dram_tensor("dbg", [128, 8, TPC], BF16,
                                     kind="ExternalOutput").ap()
                nc.sync.dma_start(out=dbg, in_=stag[:])
                break

            # ---- scatter staging -> head-major qT/kT at partitions 0:32 ----
            # Every head's 32 grouped d-rows land on partitions 0:32 (a-half
            # rows 0:16, b-half 16:32) at free offset h*TPC, so every scores
            # matmul reads its stationary from the same partition base.
            # (Cycling partial-row stationary bases across matmuls crashes
            # the device; fixed-base partial-row stationaries are fine.)
            qkT = {}
            for Ti, T in enumerate("qk"):
                qt = qkT_p.tile([128, 16, TPC], BF16, tag=T + "T")
                qkT[T] = qt
                for Hi in range(2):              # a, b halves
                    blk0 = (Ti * 2 + Hi) * 2     # Mc=0 block; Mc=1 adjacent
                    for hl in range(8):
                        src = stag[16 * hl:16 * hl + 16,
                                   blk0:blk0 + 2, :]
                        dst = qkT[T][16 * Hi:16 * Hi + 16, :, :] \
                            .rearrange("p (mc hl) t -> p mc hl t", mc=2)[
                            :, :, hl, :]
                        eng = nc.sync if (hl % 2 == 0) else nc.gpsimd
                        eng.dma_start(out=dst, in_=src)

            if debug_stop == "scatter":
                dbg = nc.dram_tensor("dbg", [2, 128, 16, TPC], BF16,
                                     kind="ExternalOutput").ap()
                nc.sync.dma_start(out=dbg[0], in_=qkT["q"][:])
                nc.sync.dma_start(out=dbg[1], in_=qkT["k"][:])
                break

            # ---- v GEMM (token-major) -> zero-padded per-window v ----
            # vz[:, g, ws, h, :] holds head h's v rows for window ws of group
            # g on that window's 64 partitions, zeros on the other 64. This
            # keeps every AV stationary full-128-partition (partial-row
            # stationaries alternating base partitions crash the device).
            vz = vz_tiles[ck % 2]
            for g in range(NG):
                ps = ps_gemm.tile([128, 512], F32, tag="gemm")
                for kc in range(4):
                    nc.tensor.matmul(
                        ps[:],
                        lhsT=xt_t[:, kc, 128 * g:128 * (g + 1)],
                        rhs=wv_sb[:, kc],
                        start=(kc == 0), stop=(kc == 3))
                psh = ps[:].rearrange("p (h e) -> p h e", h=16)
                nc.vector.tensor_copy(out=vz[0:64, g, 0], in_=psh[0:64])
                nc.vector.tensor_copy(out=vz[64:128, g, 1], in_=psh[64:128])

            if debug_stop == "vprime":
                dbg = nc.dram_tensor("dbg", [128, NG, 2, 16, 32], BF16,
                                     kind="ExternalOutput").ap()
                nc.sync.dma_start(out=dbg, in_=vz[:])
                break

            # ---- attention per (2-window group g, 4-head group G4) ----
            # 1-bank psum tiles per 4-head group, double-buffered, so scores
            # matmuls of the next group overlap exp/AV/normalize of this one
            # and the PE stream stays dense (keeps HAM un-throttled).
            ao = ao_p.tile([128, NG, 512], BF16, tag="ao")
            rcp = rc_p.tile([128, NG, 4, 8], F32, tag="rc")
            for g in range(NG):
                for G4 in range(4):
                    # scores.T : 4 heads into one (128, 512) psum (1 bank)
                    ps_s = ps_sc.tile([128, 4, 128], F32, tag="sc")
                    expg = exp_p.tile([128, 4, 128], BF16, tag="exp")
                    for jj in range(4):
                        h = 4 * G4 + jj
                        cols = slice(128 * g, 128 * (g + 1))
                        nc.tensor.matmul(
                            ps_s[:, jj],
                            lhsT=qkT["k"][0:32, h, cols],
                            rhs=qkT["q"][0:32, h, cols],
                            start=(jj == 0), stop=(jj == 3))
                    nc.scalar.activation(
                        out=expg[:], in_=ps_s[:],
                        func=mybir.ActivationFunctionType.Exp)

                    # AV + denominators, one bank: cols 0:256 v-slots,
                    # 256:264 denominator pairs. Stationary = exp(S.T), full
                    # 128 partitions; zero-padded vz / wsones mask the other
                    # window's keys.
                    ps_a = ps_av.tile([128, 512], F32, tag="av")
                    n_mm = 12
                    mm_i = 0
                    for jj in range(4):
                        h = 4 * G4 + jj
                        for ws in range(2):
                            s = jj * 2 + ws
                            nc.tensor.matmul(
                                ps_a[:, 32 * s:32 * s + 32],
                                lhsT=expg[:, jj],
                                rhs=vz[:, g, ws, h],
                                start=(mm_i == 0), stop=(mm_i == n_mm - 1))
                            mm_i += 1
                        nc.tensor.matmul(
                            ps_a[:, 256 + 2 * jj:258 + 2 * jj],
                            lhsT=expg[:, jj],
                            rhs=wsones[:],
                            start=(mm_i == 0), stop=(mm_i == n_mm - 1))
                        mm_i += 1

                    # normalize: recip of denominators, scale valid halves
                    vslots = ps_a[:, 0:256].rearrange(
                        "p (j w e) -> p j w e", j=4, w=2)
                    dens = ps_a[:, 256:264].rearrange("p (j w) -> p j w", j=4)
                    nc.vector.reciprocal(
                        out=rcp[:, g, G4].rearrange("p (j w) -> p j w", j=4),
                        in_=dens)
                    for ws in range(2):
                        src = vslots[64 * ws:64 * ws + 64, :, ws, :]
                        rin = rcp[64 * ws:64 * ws + 64, g, G4] \
                            .rearrange("p (j w) -> p j w", j=4)[:, :, ws] \
                            .broadcast_to((64, 4, 32))
                        dst = ao[64 * ws:64 * ws + 64, g,
                                 128 * G4:128 * (G4 + 1)].rearrange(
                            "p (j e) -> p j e", j=4)
                        nc.vector.tensor_tensor(
                            out=dst, in0=src, in1=rin, op=mybir.AluOpType.mult)

            # ---- transpose attnout -> d-major ----
            aoT = aoT_p.tile([128, 4, TPC], BF16, tag="aoT")
            for g in range(NG):
                ps_t = ps_tr.tile([128, 4, 128], BF16, tag="tr")
                for m in range(4):
                    nc.tensor.transpose(
                        ps_t[:, m],
                        ao[:, g, 128 * m:128 * (m + 1)],
                        ident[:])
                nc.vector.tensor_copy(
                    out=aoT[:, :, 128 * g:128 * (g + 1)], in_=ps_t[:])

            if debug_stop == "trans":
                dbg = nc.dram_tensor("dbg", [128, 4, TPC], BF16,
                                     kind="ExternalOutput").ap()
                nc.sync.dma_start(out=dbg, in_=aoT[:])
                break

            # ---- proj GEMM (d-major out) + store ----
            for Mc in range(4):
                for th in range(TPC // PW):
                    ps_o = ps_pj.tile([128, PW], F32, tag="pj")
                    for m in range(4):
                        nc.tensor.matmul(
                            ps_o[:],
                            lhsT=wp_sb[:, m, 128 * Mc:128 * Mc + 128],
                            rhs=aoT[:, m, PW * th:PW * (th + 1)],
                            start=(m == 0), stop=(m == 3))
                    ost = os_p.tile([128, PW], F32, tag="os")
                    nc.scalar.copy(out=ost[:], in_=ps_o[:])
                    nc.sync.dma_start(
                        out=out_d[Mc, :, t0 + PW * th:t0 + PW * (th + 1)],
                        in_=ost[:])

    nc.compile()
    return nc


# ---------------- host driver ----------------

_PROG_CACHE = {}


def _get_program(CH=8, NCHUNK=16):
    key = (CH, NCHUNK)
    if key not in _PROG_CACHE:
        _PROG_CACHE[key] = build_program(CH, NCHUNK)
    return _PROG_CACHE[key]


def make_in_maps(x, wp_dict, CH=8, NCHUNK=16, n_cores=N_CORES):
    xw = window_partition(np.asarray(x, np.float32))     # (1024, 64, 512)
    nt = CH * NCHUNK * NTOK
    win_per_core = nt // NTOK
    in_maps = []
    for c in range(n_cores):
        xs = xw[c * win_per_core:(c + 1) * win_per_core].reshape(nt, DIM)
        xt = np.ascontiguousarray(xs.T).astype(ml_dtypes.bfloat16)
        in_maps.append(dict(
            xt=np.ascontiguousarray(xt.reshape(4, 128, nt)),
            wqa=wp_dict["wqa"], wqb=wp_dict["wqb"],
            wka=wp_dict["wka"], wkb=wp_dict["wkb"],
            wv=wp_dict["wv"], wp=wp_dict["wp"],
        ))
    return in_maps


def _run(x, qkv_w, qkv_b, proj_w, proj_b, trace=False):
    from concourse.bass_utils import run_bass_kernel_spmd

    wp_dict = prep_weights(qkv_w, qkv_b, proj_w, proj_b)
    assert wp_dict["bias_zero"], "nonzero biases not supported by this kernel"

    nc = _get_program()
    in_maps = make_in_maps(x, wp_dict)
    res = run_bass_kernel_spmd(nc, in_maps, list(range(N_CORES)), trace=trace)

    x = np.asarray(x)
    B, H, W, C = x.shape
    outs = []
    for c in range(N_CORES):
        oT = np.asarray(res.results[c]["outT"]).reshape(DIM, NT)
        outs.append(np.ascontiguousarray(oT.T))          # (8192, 512)
    ow = np.concatenate(outs, 0).reshape(NWIN, NTOK, DIM)
    out = window_unpartition(ow, B, H, W).astype(np.float32)
    return out, res


def kernel(x, qkv_w, qkv_b, proj_w, proj_b):
    out, _ = _run(x, qkv_w, qkv_b, proj_w, proj_b, trace=False)
    return out


if __name__ == "__main__":
    build_program(2, 2)
    print("mini program built OK")
    build_program()
    print("full program built OK")
